# revision 74
# baseline (speedup 1.0000x reference)
"""AttentionBlock (ChannelNorm + MHA + proj + residual) Trainium2 Bass kernel.

Sharding: 8 cores = 4 batches x 2 head-groups. Core c handles batch c//2 and
heads [4*(c%2), 4*(c%2)+4). Each core computes LayerNorm + its slice of the
QKV projection + attention for its 4 heads + a partial proj_out contraction.
The host sums the two partials per batch and adds proj bias + residual.

Design notes
------------
The LN-normalized input is never materialized. Because the per-position
affine commutes with the channel contraction,
    W @ ((x - mu) * rstd) = (W @ x - mu * (W @ 1)) * rstd,
the QKV GEMMs run directly on raw x (so they depend only on the x DMA, not
on the stats), and the LN fixup folds into:
  - one extra K=2 accumulation matmul per output tile with stationary
    [wsum; bias] and moving [-mu; sqrt(var+eps)] rows (the bias row is
    pre-divided by rstd via the sqrt so the later rstd multiply restores it),
  - one elementwise multiply by rstd: a PE-broadcast tile for the [m, l]
    q,k layout, a per-partition scalar for the [l, d] v layout.
Host-side prep folds ln_gamma into W and ln_beta into the bias, and
precomputes the W row-sums. The v GEMM is 260 wide: each head's 65th column
has weight 0 and bias 1, so after the rstd multiply it lands at
srstd*rstd ~= 1.0 -- the softmax-denominator ones column comes out of the
GEMM for free.

Phase B: heads run in pairs (2hp, 2hp+1) whose q/k rows sit on partitions
0-63 / 64-127 of the same mc tile, so the two K=64 score matmuls row-tile
into the PE array's two halves (tile_position row groups) and execute
concurrently. Scores are computed transposed (s^T[lk, lq]); the softmax
denominator is row DH of the M=65 o-matmul; o-matmuls trail the scores by
two steps in the in-order PE FIFO so oT PSUM recycling never blocks the
pipeline head. The Scalar/ACT engine runs nothing but the 128 softmax exps
(the kernel's floor); per-head 1/Z runs on the DVE (reciprocal_approx_fast
needs raw IEEE bits, so Z is copied out of PSUM first) with the partition
broadcast done by DRAM-bounce DMA on otherwise idle DMA queues.

QKV/stats matmuls run in float32r; everything downstream of the q/k/v
fixups (scores, o, proj) runs bf16 x bf16 -> fp32 PSUM, which halves
LDWEIGHTS time and PE datapath power (the HAM power-throttle window at the
start of the attention phase is the main clock limiter).
"""
import os

import numpy as np
import ml_dtypes

import concourse.bass as bass
import concourse.mybir as mybir
import concourse.tile as tile
from concourse import bacc
from concourse import bass_utils as _bass_utils
from concourse.bass_utils import run_bass_kernel_spmd

# NOTE: walrus --enable-ldw-opt is left OFF: bf16 matmuls emit standalone
# InstLdweights, which that optimization rejects outright.

F32 = mybir.dt.float32
F32R = mybir.dt.float32r
BF16 = mybir.dt.bfloat16
NPBF16 = ml_dtypes.bfloat16

B, C, L, H = 4, 512, 2048, 8
DH = C // H          # 64
G = 2                # head groups (cores per batch)
HPC = H // G         # 4 heads per core
P = 128
KC = C // P          # 4 contraction chunks
NSTRIP = 4
STRIP = L // NSTRIP  # 512
LCH = L // P         # 16 l-chunks
DV = HPC * (DH + 1)  # 260: v GEMM width incl per-head ones columns
SCALE = DH ** -0.5
EPS = 1e-5
ALU = mybir.AluOpType
ACTF = mybir.ActivationFunctionType


def build_nc():
    nc = bacc.Bacc()
    x_d = nc.dram_tensor("x_sh", [NSTRIP, P, KC, STRIP], F32R, kind="ExternalInput")
    wqk_d = nc.dram_tensor("wqkT", [P, KC, 2 * HPC * DH], BF16, kind="ExternalInput")
    wv_d = nc.dram_tensor("wvT", [P, KC, DV], BF16, kind="ExternalInput")
    wp_d = nc.dram_tensor("wprojT", [P, 2, C], BF16, kind="ExternalInput")
    wbqk_d = nc.dram_tensor("wbqk", [2, 4, P], F32R, kind="ExternalInput")
    wbv_d = nc.dram_tensor("wbv", [2, DV], F32R, kind="ExternalInput")
    ones_d = nc.dram_tensor("ones_in", [P, P], F32R, kind="ExternalInput")
    out_d = nc.dram_tensor("out_part", [NSTRIP, P, 4, STRIP], F32,
                           kind="ExternalOutput")
    # DRAM bounce buffers: per-head 1/Z partition-broadcast, rstd transpose
    rz_d = nc.dram_tensor("rz_scr", [HPC, 4, STRIP], F32)
    rstd_d = nc.dram_tensor("rstd_scr", [NSTRIP, STRIP], F32)
    _dbg = bool(os.environ.get("BASS_DBG_QKV"))
    if _dbg:
        U16 = mybir.dt.uint16
        qkT_dbg = nc.dram_tensor("qkT_dbg", [P, 4, L], U16,
                                 kind="ExternalOutput")
        v_dbg = nc.dram_tensor("v_dbg", [P, LCH, HPC, DH + 1], U16,
                               kind="ExternalOutput")
        onT_dbg = nc.dram_tensor("onT_dbg", [P, 2, L], U16,
                                 kind="ExternalOutput")
        xb_dbg = nc.dram_tensor("xb_dbg", [P, KC, STRIP], U16,
                                kind="ExternalOutput")

    with tile.TileContext(nc) as tc:
        with (
            tc.tile_pool(name="persist", bufs=1) as pp,
            tc.tile_pool(name="small", bufs=4) as sp,
        ):
            # ---- persistent tiles ----
            ones_sb = pp.tile([P, P], F32R)             # stats/bcast ones
            wqk_sb = pp.tile([P, KC, 2 * HPC * DH], BF16)    # [128,4,512]
            wv_sb = pp.tile([P, KC, DV], BF16)               # [128,4,260]
            wp_sb = pp.tile([P, 2, C], BF16)                 # [128,2,512]
            wbqk_sb = pp.tile([2, 4, P], F32R)
            wbv_sb = pp.tile([2, DV], F32R)
            qkT_sb = pp.tile([P, 4, L], BF16)                # q^T,k^T [c_out,l]
            v_sb = pp.tile([P, LCH, HPC, DH + 1], BF16)      # v + ones col
            onT_sb = pp.tile([P, 2, L], BF16)                # normalized o^T
            eps_sb = sp.tile([1, 1], F32)

            nc.vector.memset(eps_sb[:], EPS)

            # ================= phase A: stats + QKV on raw x =================
            with (
                tc.tile_pool(name="xa", bufs=4) as xa,
                tc.tile_pool(name="x2a", bufs=2) as x2a,
                tc.tile_pool(name="stats", bufs=4) as st,
                tc.tile_pool(name="psumA", bufs=1, space="PSUM") as psA,
                tc.tile_pool(name="psumBC", bufs=2, space="PSUM") as psBC,
            ):
                # DMA order = first-use order: ones gates the stats matmuls,
                # x strips gate everything, weights come later
                nc.sync.dma_start(ones_sb[:], ones_d[:])
                x_tiles = []
                for s in range(NSTRIP):
                    x_sb = xa.tile([P, KC, STRIP], F32R, tag="x", name=f"x{s}")
                    nc.sync.dma_start(x_sb[:], x_d[s])
                    x_tiles.append(x_sb)
                nc.sync.dma_start(wbv_sb[:], wbv_d[:])
                nc.sync.dma_start(wv_sb[:], wv_d[:])
                nc.sync.dma_start(wbqk_sb[:], wbqk_d[:])
                nc.sync.dma_start(wqk_sb[:], wqk_d[:])
                nc.sync.dma_start(wp_sb[:], wp_d[:])

                # ---- per-strip stats: sum, sum-sq -> -mu / srstd / rstd ----
                xb_tiles = []
                stk_tiles = []    # [2,512]: row0 = -mu, row1 = sqrt(var+eps)
                rstdT_tiles = []  # [128,4]: rstd with l on partitions
                rb_tiles = []     # [128,512]: rstd broadcast, SBUF
                for s in range(NSTRIP):
                    x_sb = x_tiles[s]
                    x2 = x2a.tile([P, KC, STRIP], F32R, tag="x2", name=f"x2_{s}")
                    nc.scalar.activation(x2[:], x_sb.bitcast(F32)[:], ACTF.Square)
                    xb = xa.tile([P, KC, STRIP], BF16, tag="xb", name=f"xb{s}")
                    nc.scalar.copy(xb[:], x_sb.bitcast(F32)[:])
                    xb_tiles.append(xb)
                    ps_sum = psA.tile([1, STRIP], F32, tag="ssum", name=f"psum{s}")
                    ps_sq = psA.tile([1, STRIP], F32, tag="ssq", name=f"psq{s}")
                    for kc in range(KC):
                        nc.tensor.matmul(
                            ps_sum[:], ones_sb[:, 0:1], x_sb[:, kc, :],
                            start=(kc == 0), stop=(kc == KC - 1),
                        )
                    for kc in range(KC):
                        nc.tensor.matmul(
                            ps_sq[:], ones_sb[:, 0:1], x2[:, kc, :],
                            start=(kc == 0), stop=(kc == KC - 1),
                        )
                    stk = st.tile([2, STRIP], F32R, tag="stk", name=f"stk{s}")
                    rstd = st.tile([1, STRIP], F32R, tag="rstd", name=f"rstd{s}")
                    rstdT = st.tile([P, STRIP // P], F32, tag="rstdT",
                                    name=f"rstdT{s}")
                    mu2 = st.tile([1, STRIP], F32, tag="mu2", name=f"mu2_{s}")
                    var = st.tile([1, STRIP], F32, tag="var", name=f"var{s}")
                    srstd = st.tile([1, STRIP], F32, tag="srstd",
                                    name=f"srstd{s}")
                    rstd_f = st.tile([1, STRIP], F32, tag="rstdf",
                                     name=f"rstdf{s}")
                    nc.vector.tensor_scalar_mul(stk[0:1, :], ps_sum[:], -1.0 / C)
                    nc.vector.tensor_mul(mu2[:], stk.bitcast(F32)[0:1, :],
                                         stk.bitcast(F32)[0:1, :])
                    nc.vector.scalar_tensor_tensor(
                        var[:], ps_sq[:], 1.0 / C, mu2[:],
                        op0=ALU.mult, op1=ALU.subtract,
                    )
                    # engine writes must be partition-quad aligned, so Sqrt
                    # lands in a partition-0 tile and a DMA places it on
                    # partition 1 of stk
                    nc.scalar.activation(srstd[:], var[:], ACTF.Sqrt,
                                         bias=eps_sb[:])
                    nc.gpsimd.dma_start(stk[1:2, :], srstd[:])
                    nc.vector.reciprocal_approx_fast(rstd_f[:], srstd[:])
                    # matmul operands need explicit f32r rounding
                    nc.vector.tensor_copy(rstd[:], rstd_f[:])
                    # rstd transposed to [l-on-partitions, lc] for the v
                    # fixup's per-partition scalar (via DRAM: SBUF-to-SBUF
                    # partition redistribution is not reliable)
                    nc.sync.dma_start(rstd_d[s:s + 1, :], rstd_f[:])
                    nc.sync.dma_start(
                        rstdT[:],
                        rstd_d[s].rearrange("(lc p) -> p lc", p=P))
                    # rstd broadcast across partitions, bounced to SBUF
                    # (engine ops may read only one PSUM operand)
                    rstd_b = psBC.tile([P, STRIP], F32, tag="rb",
                                       name=f"rb{s}")
                    rb_sb = st.tile([P, STRIP], F32, tag="rbsb",
                                    name=f"rbsb{s}")
                    nc.tensor.matmul(rstd_b[:], ones_sb[0:1, :], rstd[:],
                                     start=True, stop=True)
                    nc.vector.tensor_copy(rb_sb[:], rstd_b[:])
                    stk_tiles.append(stk)
                    rstdT_tiles.append(rstdT)
                    rb_tiles.append(rb_sb)

                # ---- QKV GEMMs on raw x + LN fixup ----
                with (
                    tc.tile_pool(name="psumQ", bufs=2, space="PSUM") as psQ,
                    tc.tile_pool(name="psumV", bufs=2, space="PSUM") as psV,
                ):
                    # v GEMM first: out [l, d+ones]; LN fixup terms enter via
                    # a K=2 accumulation matmul, rstd applies as a
                    # per-partition scalar (l is on partitions here)
                    for s in range(NSTRIP):
                        stk = stk_tiles[s]
                        rstdT = rstdT_tiles[s]
                        for lc in range(STRIP // P):
                            lg = s * (STRIP // P) + lc
                            pv = psV.tile([P, DV], F32, tag="v",
                                          name=f"pv{s}_{lc}")
                            for kc in range(KC):
                                nc.tensor.matmul(
                                    pv[:], xb_tiles[s][:, kc, bass.ts(lc, P)],
                                    wv_sb[:, kc, :],
                                    start=(kc == 0), stop=False,
                                )
                            nc.tensor.matmul(
                                pv[:], stk[:, bass.ts(lc, P)], wbv_sb[:],
                                start=False, stop=True,
                            )
                            nc.vector.tensor_scalar(
                                v_sb[:, lg, :, :],
                                pv.rearrange("p (h d) -> p h d", h=HPC),
                                rstdT[:, lc:lc + 1], None, op0=ALU.mult,
                            )

                    # q^T,k^T GEMM: out [c_out, l]; mc-major with order
                    # 0,2,1,3 so heads 0/1's q and k complete first
                    for mc in (0, 2, 1, 3):
                        for s in range(NSTRIP):
                            ls = bass.ts(s, STRIP)
                            pqk = psQ.tile([P, STRIP], F32, tag="qk",
                                           name=f"pqk{s}_{mc}")
                            for kc in range(KC):
                                nc.tensor.matmul(
                                    pqk[:], wqk_sb[:, kc, bass.ts(mc, P)],
                                    xb_tiles[s][:, kc, :],
                                    start=(kc == 0), stop=False,
                                )
                            nc.tensor.matmul(
                                pqk[:], wbqk_sb[:, mc, :], stk_tiles[s][:],
                                start=False, stop=True,
                            )
                            nc.vector.tensor_mul(qkT_sb[:, mc, ls], pqk[:],
                                                 rb_tiles[s][:])

            if _dbg:
                nc.sync.dma_start(qkT_dbg[:],
                                  qkT_sb.bitcast(mybir.dt.uint16)[:],)
                nc.sync.dma_start(xb_dbg[:],
                                  xb_tiles[0].bitcast(mybir.dt.uint16)[:])

            # ================= phase B: attention per head =================
            with (
                tc.tile_pool(name="expp", bufs=6) as ep,
                tc.tile_pool(name="rdout", bufs=2) as ro,
                tc.tile_pool(name="psumB", bufs=2, space="PSUM") as psB,
                tc.tile_pool(name="psumO", bufs=4, space="PSUM") as psO,
            ):
                # Heads run in PAIRS (2hp, 2hp+1): their q/k live on
                # partitions 0-63 / 64-127 of the same mc tile, so the two
                # K=64 score matmuls row-tile into the PE array's two halves
                # and execute concurrently (tile_position row groups).
                # L is processed in halves so a block's four oT accumulators
                # fit in 4 PSUM banks. The whole phase is one flat software
                # pipeline: o-matmuls trail the scores by DEFER steps ACROSS
                # block boundaries, so the normalize's DRAM-bounce latency
                # (which gates the next block's oT allocations) hides under
                # the next block's score stream instead of stalling the
                # in-order PE FIFO.
                DEFER = 2
                blocks = [(hp, s2) for hp in range(2) for s2 in range(2)]
                oT_all = {}
                ex_all = {}

                def emit_scores(hp, s2, lk):
                    ex = ep.tile([P, 2, 1024], BF16, tag="expT")
                    ex_all[hp, s2, lk] = ex
                    pst = [psB.tile([P, 1024], F32, tag="sT",
                                    name=f"sT{hp}{s2}{lk}_{hi}")
                           for hi in range(2)]
                    for q2 in range(2):
                        for hi in range(2):
                            po = hi * DH
                            nc.tensor.matmul(
                                pst[hi][:, bass.ts(q2, 512)],
                                qkT_sb[po:po + DH, 2 + hp, bass.ts(lk, P)],
                                qkT_sb[po:po + DH, hp,
                                       bass.ds(s2 * 1024 + q2 * 512, 512)],
                                start=True, stop=True,
                                tile_position=(po, 0),
                            )
                    for hi in range(2):
                        nc.scalar.activation(
                            ex[:, hi, :], pst[hi][:], ACTF.Exp, scale=SCALE,
                        )

                def emit_o(hp, s2, lk):
                    if lk == 0:
                        oT_all[hp, s2] = {
                            (hi, st): psO.tile([DH + 1, STRIP], F32, tag="oT",
                                               name=f"oT{hp}{s2}_{hi}{st}")
                            for hi in range(2) for st in range(2)}
                    oT = oT_all[hp, s2]
                    exo = ex_all.pop((hp, s2, lk))
                    for hi in range(2):
                        for st in range(2):
                            nc.tensor.matmul(
                                oT[hi, st][:], v_sb[:, lk, 2 * hp + hi, :],
                                exo[:, hi, bass.ts(st, 512)],
                                start=(lk == 0), stop=(lk == LCH - 1),
                            )
                    if lk == LCH - 1:
                        emit_normalize(hp, s2)

                def emit_normalize(hp, s2):
                    # onT[d, l] = oT[d, l] / Z[l]; 1/Z on DVE, broadcast
                    # across partitions via DRAM bounce on idle DMA queues
                    oT = oT_all.pop((hp, s2))
                    for hi in range(2):
                        h = 2 * hp + hi
                        po = hi * DH
                        for st in range(2):
                            s = 2 * s2 + st
                            rz_f = ro.tile([1, STRIP], F32, tag="rzf",
                                           bufs=4, name=f"rzf{h}_{s}")
                            rz_b = ro.tile([DH, STRIP], F32, tag="rzb",
                                           bufs=4, name=f"rzb{h}_{s}")
                            zrow = ro.tile([1, STRIP], F32, tag="zrow",
                                           bufs=4, name=f"zrow{h}_{s}")
                            # reciprocal_approx_fast is a bit-trick op and
                            # must read raw IEEE bits from SBUF, not PSUM
                            nc.vector.tensor_copy(zrow[:],
                                                  oT[hi, st][DH:DH + 1, :])
                            nc.vector.reciprocal_approx_fast(rz_f[:], zrow[:])
                            nc.sync.dma_start(rz_d[h, s:s + 1, :], rz_f[:])
                            nc.sync.dma_start(
                                rz_b[:],
                                rz_d[h, s:s + 1, :]
                                .partition_broadcast(DH).opt())
                            nc.vector.tensor_mul(
                                onT_sb[po:po + DH, hp, bass.ts(s, STRIP)],
                                oT[hi, st][0:DH, :], rz_b[:],
                            )

                for hp, s2 in blocks:
                    for lk in range(LCH):
                        emit_scores(hp, s2, lk)
                        if lk >= DEFER:
                            emit_o(hp, s2, lk - DEFER)
                    for lk in range(LCH - DEFER, LCH):
                        emit_o(hp, s2, lk)

                if _dbg:
                    nc.sync.dma_start(onT_dbg[:],
                                      onT_sb.bitcast(mybir.dt.uint16)[:])
                    nc.sync.dma_start(v_dbg[:],
                                      v_sb.bitcast(mybir.dt.uint16)[:])

                # ============ phase C: proj partial ============
                # proj PSUM reuses the sT tag slots (no pool-close barrier, so
                # proj matmuls overlap the tail of the last head); PSUM->SBUF
                # copies split between the now-idle ACT and DVE engines.
                for s in range(NSTRIP):
                    ls = bass.ts(s, STRIP)
                    ot = ro.tile([P, 4, STRIP], F32, tag="out", bufs=2,
                                 name=f"out{s}")
                    for mc in range(4):
                        ppj = psB.tile([P, STRIP], F32, tag="sT",
                                       name=f"proj{s}_{mc}")
                        for kc in range(2):
                            nc.tensor.matmul(
                                ppj[:], wp_sb[:, kc, bass.ts(mc, P)],
                                onT_sb[:, kc, ls],
                                start=(kc == 0), stop=(kc == 1),
                            )
                        if mc % 2 == 0:
                            nc.scalar.copy(ot[:, mc, :], ppj[:])
                        else:
                            nc.vector.tensor_copy(ot[:, mc, :], ppj[:])
                    nc.sync.dma_start(out_d[s], ot[:])

    nc.compile()
    return nc


_NC = None


def _get_nc():
    global _NC
    if _NC is None:
        _NC = build_nc()
    return _NC


def make_core_inputs(x, ln_gamma, ln_beta, w_qkv, b_qkv, w_proj, b_proj):
    """Host-side shard prep. Folds ln_gamma/ln_beta into the QKV weights and
    biases, precomputes the weight row-sums for the LN-commute fixup, and
    lays every tensor out in its exact SBUF shape (contiguous DMAs)."""
    x = np.asarray(x, np.float32)
    g_ = np.asarray(ln_gamma, np.float32)
    be = np.asarray(ln_beta, np.float32)
    w_qkv = np.asarray(w_qkv, np.float32)
    b_qkv = np.asarray(b_qkv, np.float32)
    w_proj = np.asarray(w_proj, np.float32)

    def sb_layout(m):  # [K, M] -> [P, K//P, M]
        return np.ascontiguousarray(
            m.reshape(m.shape[0] // P, P, m.shape[1]).transpose(1, 0, 2))

    in_maps = []
    for core in range(8):
        b = core // 2
        gr = core % 2
        rs = slice(gr * HPC * DH, (gr + 1) * HPC * DH)
        wq, wk, wv = (w_qkv[i * C:(i + 1) * C][rs] for i in range(3))
        bq, bk, bv = (b_qkv[i * C:(i + 1) * C][rs] for i in range(3))
        # gamma folds into W columns; beta folds into the bias
        wqg, wkg, wvg = (w * g_[None, :] for w in (wq, wk, wv))
        bq = bq + wq @ be
        bk = bk + wk @ be
        bv = bv + wv @ be
        wqk = np.concatenate([wqg, wkg], 0)          # [512, 512]
        bqk = np.concatenate([bq, bk])               # [512]
        wsqk = wqk.sum(1)                            # [512]
        wbqk = np.stack([wsqk.reshape(4, P), bqk.reshape(4, P)], 0)
        # v extended to 260 columns: per-head 65th column has weight 0 and
        # bias 1 -> becomes the softmax-denominator ones column
        wv_ext = np.zeros((C, DV), np.float32)       # [512, 260]
        bv_ext = np.zeros(DV, np.float32)
        wsv_ext = np.zeros(DV, np.float32)
        for hh in range(HPC):
            wv_ext[:, hh * (DH + 1):hh * (DH + 1) + DH] = \
                wvg.T[:, hh * DH:(hh + 1) * DH]
            bv_ext[hh * (DH + 1):hh * (DH + 1) + DH] = \
                bv[hh * DH:(hh + 1) * DH]
            bv_ext[hh * (DH + 1) + DH] = 1.0
            wsv_ext[hh * (DH + 1):hh * (DH + 1) + DH] = \
                wvg.sum(1)[hh * DH:(hh + 1) * DH]
        wbv = np.stack([wsv_ext, bv_ext], 0)         # [2, 260]
        # x in strip-major SBUF shape [NSTRIP, P, KC, STRIP]
        xs = (x[b].reshape(KC, P, NSTRIP, STRIP).transpose(2, 1, 0, 3))
        in_maps.append({
            "x_sh": np.ascontiguousarray(xs),
            "wqkT": sb_layout(wqk.T).astype(NPBF16),
            "wvT": sb_layout(wv_ext).astype(NPBF16),
            "wprojT": sb_layout(w_proj[:, rs].T).astype(NPBF16),
            "wbqk": np.ascontiguousarray(wbqk),
            "wbv": np.ascontiguousarray(wbv),
            "ones_in": np.ones((P, P), np.float32),
        })
    return in_maps


def combine(partials, x, b_proj):
    out = np.empty((B, C, L), np.float32)
    for b in range(B):
        # partial [NSTRIP, P, 4, STRIP] -> [C, L]
        p = (np.asarray(partials[2 * b]) + np.asarray(partials[2 * b + 1]))
        p = p.transpose(2, 1, 0, 3).reshape(C, L)
        out[b] = p + np.asarray(b_proj, np.float32)[:, None] \
            + np.asarray(x, np.float32)[b]
    return out


def run_cores(in_maps, trace=False, **kw):
    nc = _get_nc()
    return run_bass_kernel_spmd(nc, in_maps, core_ids=list(range(8)),
                                trace=trace, **kw)


def kernel(**inputs):
    in_maps = make_core_inputs(**inputs)
    res = run_cores(in_maps)
    partials = [r["out_part"] for r in res.results]
    return combine(partials, inputs["x"], inputs["b_proj"])
